# revision 2
# baseline (speedup 1.0000x reference)
"""Trainium2 Bass kernel for RangeLinearQuantParamLayerWrapper (symmetric int8
quantized linear: y = dequant(requant(x_q @ W_q.T + b_q))).

Full inputs in, full output out. Internally sharded over 8 NeuronCores on a
4 (batch) x 2 (out_features) grid:
  - x^T shard  [4096 i, 1024 b]  per core (batch cols)
  - W^T shard  [16 m-blocks, 4096 i, 128 o] per core (out cols)
  - absmax slices of x / W (1/8 each per core) + AllReduce(max) for scales
  - bf16 matmul (quantized values are exact small ints in bf16), f32 PSUM
  - second AllReduce(max) over the int32 accumulator for the output scale
Output per core: out_f^T block [2048 o, 1024 b], transposed+assembled on host.
"""
import sys

sys.path.insert(0, "/opt/trn_rl_repo")
import numpy as np

NCORES = 8
GR, GC = 4, 2          # core grid: 4 batch groups x 2 out-feature groups
B = O = K = 4096
BS = B // GR           # 1024 batch cols per core
OS = O // GC           # 2048 out cols per core
MT = OS // 128         # 16 o-blocks per core
KT = K // 128          # 32 k tiles
NBT = BS // 512        # 2 b panels per o-block
C_MAGIC = 1.5 * 2.0**23          # f32 add/sub constant => round-to-nearest-even
C_P127 = C_MAGIC + 127.0

_CACHE = {}


def _build_nc():
    import concourse.bass as bass
    import concourse.mybir as mybir
    import concourse.tile as tile
    from concourse import bacc, bass_isa

    f32 = mybir.dt.float32
    bf16 = mybir.dt.bfloat16
    Alu = mybir.AluOpType
    Act = mybir.ActivationFunctionType

    nc = bacc.Bacc("TRN2", target_bir_lowering=False, debug=False,
                   num_devices=NCORES)

    xt_d = nc.dram_tensor("xt", [K, BS], f32, kind="ExternalInput")
    wt_d = nc.dram_tensor("wt", [MT, K, 128], f32, kind="ExternalInput")
    xa_d = nc.dram_tensor("xa", [K // NCORES, K], f32, kind="ExternalInput")
    wa_d = nc.dram_tensor("wa", [K // NCORES, K], f32, kind="ExternalInput")
    bp_d = nc.dram_tensor("bp", [128, 32], f32, kind="ExternalInput")
    out_d = nc.dram_tensor("out", [OS, BS], f32, kind="ExternalOutput")

    with tile.TileContext(nc) as tc:
        with (
            tc.tile_pool(name="pers", bufs=1) as pers,
            tc.tile_pool(name="psum", bufs=4, space="PSUM") as psum,
            tc.tile_pool(name="dram", bufs=1, space="DRAM") as dram,
            tc.tile_pool(name="stat", bufs=2) as stat,
        ):
            # ---------------- phase A: local absmax ----------------
            mx = pers.tile([128, 1], f32, tag="mx")
            mw = pers.tile([128, 1], f32, tag="mw")
            nc.vector.memset(mx[:], 0.0)
            nc.vector.memset(mw[:], 0.0)

            bp = pers.tile([128, 32], f32, tag="bp")
            nc.sync.dma_start(bp[:], bp_d.ap())
            mb = pers.tile([128, 1], f32, tag="mb")
            nc.vector.tensor_reduce(
                mb[:], bp[:], axis=mybir.AxisListType.X, op=Alu.max,
                apply_absolute_value=True)

            with tc.tile_pool(name="absp", bufs=3) as absp:
                for src, acc in ((xa_d, mx), (wa_d, mw)):
                    for t in range(4):
                        for h in range(2):
                            at = absp.tile([128, 2048], f32, tag="abst")
                            nc.sync.dma_start(
                                at[:],
                                src.ap()[128 * t:128 * (t + 1),
                                         2048 * h:2048 * (h + 1)])
                            rt = absp.tile([128, 1], f32, tag="absr")
                            nc.vector.tensor_reduce(
                                rt[:], at[:], axis=mybir.AxisListType.X,
                                op=Alu.max, apply_absolute_value=True)
                            nc.vector.tensor_max(acc[:], acc[:], rt[:])

            stk = pers.tile([128, 4], f32, tag="stk")
            nc.vector.memset(stk[:], 0.0)
            nc.vector.tensor_copy(stk[:, 0:1], mx[:])
            nc.vector.tensor_copy(stk[:, 1:2], mw[:])
            nc.vector.tensor_copy(stk[:, 2:3], mb[:])
            par = pers.tile([128, 4], f32, tag="par")
            nc.gpsimd.partition_all_reduce(
                par[:], stk[:], channels=128, reduce_op=bass_isa.ReduceOp.max)

            arin = pers.tile([1, 8], f32, tag="arin")
            nc.vector.memset(arin[:], 0.0)
            nc.vector.tensor_copy(arin[0:1, 0:2], par[0:1, 0:2])
            cin = dram.tile([1, 8], f32, tag="cin")
            cout = dram.tile([1, 8], f32, tag="cout")
            nc.sync.dma_start(cin[:], arin[:])
            nc.gpsimd.collective_compute(
                "AllReduce", Alu.max,
                replica_groups=[list(range(NCORES))],
                ins=[cin[:].opt()], outs=[cout[:].opt()])
            gm = pers.tile([1, 8], f32, tag="gm")
            nc.sync.dma_start(gm[:], cout[:])

            # ---------------- scalars after AR1 ----------------
            def s255(src_ap, tagb):
                # 255 / (2*src), on partition 0
                t2 = pers.tile([1, 1], f32, tag=tagb + "t2")
                nc.vector.tensor_scalar(out=t2[:], in0=src_ap, scalar1=2.0,
                                        scalar2=None, op0=Alu.mult)
                rc = pers.tile([1, 1], f32, tag=tagb + "rc")
                nc.vector.reciprocal(rc[:], t2[:])
                s = pers.tile([1, 1], f32, tag=tagb + "s")
                nc.vector.tensor_scalar(out=s[:], in0=rc[:], scalar1=255.0,
                                        scalar2=None, op0=Alu.mult)
                return s

            sx = s255(gm[0:1, 0:1], "sx")      # in_scale
            sw = s255(gm[0:1, 1:2], "sw")      # w_scale
            sb = s255(par[0:1, 2:3], "sb")     # b_scale
            asc = pers.tile([1, 1], f32, tag="asc")        # accum_scale
            nc.vector.tensor_mul(asc[:], sx[:], sw[:])
            rbs = pers.tile([1, 1], f32, tag="rbs")
            nc.vector.reciprocal(rbs[:], sb[:])
            fb = pers.tile([1, 1], f32, tag="fb")          # accum_scale/b_scale
            nc.vector.tensor_mul(fb[:], asc[:], rbs[:])

            scal = pers.tile([1, 4], f32, tag="scal")
            nc.vector.memset(scal[:], 0.0)
            nc.vector.tensor_copy(scal[0:1, 0:1], sx[:])
            nc.vector.tensor_copy(scal[0:1, 1:2], sw[:])
            nc.vector.tensor_copy(scal[0:1, 2:3], sb[:])
            nc.vector.tensor_copy(scal[0:1, 3:4], fb[:])
            scb = pers.tile([128, 4], f32, tag="scb")
            nc.gpsimd.partition_broadcast(scb[:], scal[:], channels=128)

            cbias = pers.tile([128, 1], f32, tag="cbias")
            nc.vector.memset(cbias[:], C_MAGIC)

            # ---------------- b quantize ----------------
            bq1 = pers.tile([128, 32], f32, tag="bq1")
            nc.vector.tensor_scalar(out=bq1[:], in0=bp[:],
                                    scalar1=scb[:, 2:3], scalar2=C_MAGIC,
                                    op0=Alu.mult, op1=Alu.add)
            bq2 = pers.tile([128, 32], f32, tag="bq2")
            nc.vector.tensor_scalar(out=bq2[:], in0=bq1[:],
                                    scalar1=C_MAGIC, scalar2=127.0,
                                    op0=Alu.subtract, op1=Alu.min)
            bq3 = pers.tile([128, 16], f32, tag="bq3")
            nc.vector.tensor_scalar(out=bq3[:], in0=bq2[:, 0:16],
                                    scalar1=scb[:, 3:4], scalar2=C_MAGIC,
                                    op0=Alu.mult, op1=Alu.add)
            bqf = pers.tile([128, 16], f32, tag="bqf")
            nc.vector.tensor_scalar(out=bqf[:], in0=bq3[:],
                                    scalar1=C_MAGIC, scalar2=None,
                                    op0=Alu.subtract)

            # ---------------- quantize + matmul ----------------
            accs = []
            macc = pers.tile([128, 1], f32, tag="macc")
            with (
                tc.tile_pool(name="wsp", bufs=2) as wsp,
                tc.tile_pool(name="wt1p", bufs=2) as wt1p,
                tc.tile_pool(name="wqp", bufs=2) as wqp,
                tc.tile_pool(name="xsp", bufs=3) as xsp,
                tc.tile_pool(name="xt1p", bufs=2) as xt1p,
            ):
                KH = KT // 2

                def quant_w(m):
                    wqm = wqp.tile([128, KT, 128], bf16, tag="wq")
                    src = wt_d.ap()[m].rearrange("(k p) j -> p k j", p=128)
                    for h in range(2):
                        ws = wsp.tile([128, KH, 128], f32, tag="ws")
                        nc.sync.dma_start(ws[:], src[:, KH * h:KH * (h + 1), :])
                        wt1 = wt1p.tile([128, KH, 128], f32, tag="wt1")
                        nc.scalar.activation(wt1[:], ws[:], Act.Identity,
                                             bias=cbias[:, 0:1],
                                             scale=scb[:, 1:2])
                        nc.vector.tensor_scalar(
                            out=wqm[:, KH * h:KH * (h + 1), :], in0=wt1[:],
                            scalar1=C_MAGIC, scalar2=127.0,
                            op0=Alu.subtract, op1=Alu.min)
                    return wqm

                wq_cur = quant_w(0)

                xq = []
                for k in range(KT):
                    xs = xsp.tile([128, BS], f32, tag="xs")
                    nc.sync.dma_start(
                        xs[:], xt_d.ap()[128 * k:128 * (k + 1), :])
                    xt1 = xt1p.tile([128, BS], f32, tag="xt1")
                    nc.scalar.activation(xt1[:], xs[:], Act.Identity,
                                         bias=cbias[:, 0:1],
                                         scale=scb[:, 0:1])
                    xqk = pers.tile([128, BS], bf16, tag=f"xq{k}")
                    nc.vector.tensor_scalar(out=xqk[:], in0=xt1[:],
                                            scalar1=C_MAGIC, scalar2=127.0,
                                            op0=Alu.subtract, op1=Alu.min)
                    xq.append(xqk)

                for m in range(MT):
                    acc_m = pers.tile([128, BS], f32, tag=f"acc{m}")
                    for n in range(NBT):
                        ps = psum.tile([128, 512], f32, tag="ps")
                        for k in range(KT):
                            nc.tensor.matmul(
                                ps[:], wq_cur[:, k, :],
                                xq[k][:, 512 * n:512 * (n + 1)],
                                start=(k == 0), stop=(k == KT - 1))
                        nc.scalar.activation(
                            acc_m[:, 512 * n:512 * (n + 1)], ps[:],
                            Act.Identity, bias=bqf[:, m:m + 1], scale=1.0)
                    if m + 1 < MT:
                        wq_cur = quant_w(m + 1)
                    rt = stat.tile([128, 1], f32, tag="accr")
                    nc.vector.tensor_reduce(
                        rt[:], acc_m[:], axis=mybir.AxisListType.X,
                        op=Alu.max, apply_absolute_value=True)
                    if m == 0:
                        nc.vector.tensor_copy(macc[:], rt[:])
                    else:
                        nc.vector.tensor_max(macc[:], macc[:], rt[:])
                    accs.append(acc_m)

            # ---------------- AR2 + output scalars ----------------
            par2 = pers.tile([128, 1], f32, tag="par2")
            nc.gpsimd.partition_all_reduce(
                par2[:], macc[:], channels=128,
                reduce_op=bass_isa.ReduceOp.max)
            arin2 = pers.tile([1, 8], f32, tag="arin2")
            nc.vector.memset(arin2[:], 0.0)
            nc.vector.tensor_copy(arin2[0:1, 0:1], par2[0:1, 0:1])
            cin2 = dram.tile([1, 8], f32, tag="cin2")
            cout2 = dram.tile([1, 8], f32, tag="cout2")
            nc.sync.dma_start(cin2[:], arin2[:])
            nc.gpsimd.collective_compute(
                "AllReduce", Alu.max,
                replica_groups=[list(range(NCORES))],
                ins=[cin2[:].opt()], outs=[cout2[:].opt()])
            gm2 = pers.tile([1, 8], f32, tag="gm2")
            nc.sync.dma_start(gm2[:], cout2[:])

            rac = pers.tile([1, 1], f32, tag="rac")    # 1/accum_scale
            nc.vector.reciprocal(rac[:], asc[:])
            osat = pers.tile([1, 1], f32, tag="osat")  # out_sat
            nc.vector.tensor_mul(osat[:], gm2[0:1, 0:1], rac[:])
            oscale = s255(osat[0:1, 0:1], "os")        # out_scale
            rq = pers.tile([1, 1], f32, tag="rq")      # out_scale/accum_scale
            nc.vector.tensor_mul(rq[:], oscale[:], rac[:])
            ios = pers.tile([1, 1], f32, tag="ios")    # 1/out_scale
            nc.vector.reciprocal(ios[:], oscale[:])

            scal2 = pers.tile([1, 4], f32, tag="scal2")
            nc.vector.memset(scal2[:], 0.0)
            nc.vector.tensor_copy(scal2[0:1, 0:1], rq[:])
            nc.vector.tensor_copy(scal2[0:1, 1:2], ios[:])
            scb2 = pers.tile([128, 4], f32, tag="scb2")
            nc.gpsimd.partition_broadcast(scb2[:], scal2[:], channels=128)

            # ---------------- epilogue: requant + dequant ----------------
            with tc.tile_pool(name="epip", bufs=2) as epip:
                for m in range(MT):
                    e1 = epip.tile([128, BS], f32, tag="e1")
                    nc.vector.tensor_scalar(out=e1[:], in0=accs[m][:],
                                            scalar1=scb2[:, 0:1],
                                            scalar2=C_MAGIC,
                                            op0=Alu.mult, op1=Alu.add)
                    e2 = epip.tile([128, BS], f32, tag="e2")
                    nc.vector.tensor_scalar(out=e2[:], in0=e1[:],
                                            scalar1=C_P127, scalar2=C_MAGIC,
                                            op0=Alu.min, op1=Alu.subtract)
                    of = epip.tile([128, BS], f32, tag="of")
                    nc.scalar.activation(of[:], e2[:], Act.Copy, bias=0.0,
                                         scale=scb2[:, 1:2])
                    nc.sync.dma_start(
                        out_d.ap()[128 * m:128 * (m + 1), :], of[:])

    nc.compile()
    return nc


def _prep_inputs(x, W, b):
    xT = np.ascontiguousarray(x.T)      # [i, b]
    WT = np.ascontiguousarray(W.T)      # [i, o]
    bfull = np.ascontiguousarray(b.reshape(32, 128).T)  # [128, 32]
    sl = K // NCORES
    in_maps = []
    for core in range(NCORES):
        r, c = divmod(core, GC)
        cols = list(range(16 * c, 16 * c + 16))
        cols += [j for j in range(32) if j not in cols]
        in_maps.append({
            "xt": np.ascontiguousarray(xT[:, r * BS:(r + 1) * BS]),
            "wt": np.ascontiguousarray(
                WT[:, c * OS:(c + 1) * OS].reshape(K, MT, 128)
                .transpose(1, 0, 2)),
            "xa": xT[sl * core:sl * (core + 1), :],
            "wa": WT[sl * core:sl * (core + 1), :],
            "bp": np.ascontiguousarray(bfull[:, cols]),
        })
    return in_maps


def kernel(x, W, b):
    from concourse import bass_utils

    x = np.asarray(x, dtype=np.float32)
    W = np.asarray(W, dtype=np.float32)
    b = np.asarray(b, dtype=np.float32)
    assert x.shape == (B, K) and W.shape == (O, K) and b.shape == (O,)

    if "nc" not in _CACHE:
        _CACHE["nc"] = _build_nc()
    nc = _CACHE["nc"]

    in_maps = _prep_inputs(x, W, b)
    res = bass_utils.run_bass_kernel_spmd(
        nc, in_maps, core_ids=list(range(NCORES)))
    _CACHE["last_results"] = res

    full = np.empty((B, O), dtype=np.float32)
    for core in range(NCORES):
        r, c = divmod(core, GC)
        blk = res.results[core]["out"]          # [OS, BS] = [o, b]
        full[r * BS:(r + 1) * BS, c * OS:(c + 1) * OS] = blk.T
    return full


# revision 18
# speedup vs baseline: 122.8468x; 122.8468x over previous
"""Trainium2 Bass kernel for RangeLinearQuantParamLayerWrapper (symmetric int8
quantized linear: y = dequant(requant(x_q @ W_q.T + b_q))).

Full inputs in, full output out. Sharded over 8 NeuronCores on a
4 (batch) x 2 (out_features) grid:
  - x^T shard  [4096 i, 1024 b]  per core (batch cols)
  - W^T shard  [16 m-blocks, 4096 i, 128 o] per core (out cols)
  - per-core k-axis roll so the global max-abs scan is k-tiles [0:16) of x^T
    and k-subtiles [0:8) of each W m-block (disjoint across cores, union =
    full tensors); AllReduce(max) x2 (W first, then x) for the input scales
  - bf16 matmul (quantized values are exact small ints in bf16), f32 PSUM,
    m-blocks processed in pairs with k outermost so PE keeps pace with the
    x-quantization stream
  - third AllReduce(max) over the int32 accumulator for the output scale
Output per core: out_f^T block [2048 o, 1024 b], transposed+assembled on host.
"""
import sys

sys.path.insert(0, "/opt/trn_rl_repo")
import numpy as np

NCORES = 8
GR, GC = 4, 2          # core grid: 4 batch groups x 2 out-feature groups
B = O = K = 4096
BS = B // GR           # 1024 batch cols per core
OS = O // GC           # 2048 out cols per core
MT = OS // 128         # 16 o-blocks per core
KT = K // 128          # 32 k tiles
KHALF = KT // 2        # x-absmax k-tiles per core
NSTAGE = 12            # of those, how many stay staged in SBUF for quantize
C_MAGIC = 1.5 * 2.0**23          # f32 add/sub constant => round-to-nearest-even
C_P127 = C_MAGIC + 127.0

_CACHE = {}


def _roll_tiles(core):
    r, c = divmod(core, GC)
    return (8 * r + 16 * c) % KT


def _build_nc(sim_single_core=False):
    import concourse.bass as bass
    import concourse.mybir as mybir
    import concourse.tile as tile
    from concourse import bacc, bass_isa

    f32 = mybir.dt.float32
    bf16 = mybir.dt.bfloat16
    Alu = mybir.AluOpType
    Act = mybir.ActivationFunctionType

    nc = bacc.Bacc("TRN2", target_bir_lowering=False, debug=False,
                   num_devices=1 if sim_single_core else NCORES)

    def all_reduce_max(cin_ap, cout_ap):
        if sim_single_core:
            nc.sync.dma_start(cout_ap, cin_ap)
        else:
            nc.gpsimd.collective_compute(
                "AllReduce", mybir.AluOpType.max,
                replica_groups=[list(range(NCORES))],
                ins=[cin_ap.opt()], outs=[cout_ap.opt()])

    xt_d = nc.dram_tensor("xt", [K, BS], f32, kind="ExternalInput")
    wt_d = nc.dram_tensor("wt", [MT, K, 128], f32, kind="ExternalInput")
    bp_d = nc.dram_tensor("bp", [128, 32], f32, kind="ExternalInput")
    out_d = nc.dram_tensor("out", [OS, BS], f32, kind="ExternalOutput")

    with tile.TileContext(nc) as tc:
        with (
            tc.tile_pool(name="pers", bufs=1) as pers,
            tc.tile_pool(name="psum", bufs=8, space="PSUM") as psum,
            tc.tile_pool(name="dram", bufs=1, space="DRAM") as dram,
            tc.tile_pool(name="stat", bufs=2) as stat,
        ):
            xf_ctx = tc.tile_pool(name="xfp", bufs=1)
            xfp = xf_ctx.__enter__()

            # ------------- phase A: local absmax (k-rolled slices) -------
            mx = pers.tile([128, 1], f32, tag="mx")
            mw = pers.tile([128, 1], f32, tag="mw")
            nc.vector.memset(mx[:], 0.0)
            nc.vector.memset(mw[:], 0.0)

            bp = pers.tile([128, 32], f32, tag="bp")
            nc.sync.dma_start(bp[:], bp_d.ap())
            mb = pers.tile([128, 1], f32, tag="mb")
            nc.vector.tensor_reduce(
                mb[:], bp[:], axis=mybir.AxisListType.X, op=Alu.max,
                apply_absolute_value=True)

            wq_absp = tc.tile_pool(name="wabs", bufs=3)
            wabs = wq_absp.__enter__()

            # W absmax first: k-subtiles [0:8) of every m-block
            for m in range(MT):
                wa = wabs.tile([128, 8, 128], f32, tag="wa")
                nc.sync.dma_start(
                    wa[:],
                    wt_d.ap()[m].rearrange("(k p) j -> p k j", p=128)
                    [:, 0:8, :])
                rw = stat.tile([128, 1], f32, tag="absw")
                nc.vector.tensor_reduce(
                    rw[:], wa[:], axis=mybir.AxisListType.XY, op=Alu.max,
                    apply_absolute_value=True)
                nc.vector.tensor_max(mw[:], mw[:], rw[:])

            # early collective for w_scale (hidden under the x absmax DMA)
            parw = pers.tile([128, 1], f32, tag="parw")
            nc.gpsimd.partition_all_reduce(
                parw[:], mw[:], channels=128, reduce_op=bass_isa.ReduceOp.max)
            cinw = dram.tile([1, 8], f32, tag="cinw")
            coutw = dram.tile([1, 8], f32, tag="coutw")
            nc.sync.dma_start(cinw[0:1, 0:1], parw[0:1, 0:1])
            all_reduce_max(cinw[0:1, 0:4], coutw[0:1, 0:4])
            gmw = pers.tile([1, 8], f32, tag="gmw")
            nc.sync.dma_start(gmw[:], coutw[:])
            # sw = 255/(2*gw)
            swt = pers.tile([1, 4], f32, tag="swt")
            nc.vector.tensor_scalar(out=swt[0:1, 0:1], in0=gmw[0:1, 0:1],
                                    scalar1=2.0, scalar2=None, op0=Alu.mult)
            nc.vector.reciprocal(swt[0:1, 1:2], swt[0:1, 0:1])
            nc.vector.tensor_scalar(out=swt[0:1, 2:3], in0=swt[0:1, 1:2],
                                    scalar1=255.0, scalar2=None, op0=Alu.mult)
            scbw = pers.tile([128, 1], f32, tag="scbw")
            nc.gpsimd.partition_broadcast(scbw[:], swt[0:1, 2:3], channels=128)

            # x absmax: k-tiles [0:KHALF); first NSTAGE stay staged for reuse
            xf = []
            for k in range(KHALF):
                if k < NSTAGE:
                    t = xfp.tile([128, BS], f32, tag=f"xf{k}")
                else:
                    t = wabs.tile([128, BS], f32, tag="xtr")
                nc.sync.dma_start(t[:], xt_d.ap()[128 * k:128 * (k + 1), :])
                if k < NSTAGE:
                    xf.append(t)
                rt = stat.tile([128, 1], f32, tag="absr")
                nc.vector.tensor_reduce(
                    rt[:], t[:], axis=mybir.AxisListType.X, op=Alu.max,
                    apply_absolute_value=True)
                nc.vector.tensor_max(mx[:], mx[:], rt[:])
            wq_absp.__exit__(None, None, None)

            cbias = pers.tile([128, 1], f32, tag="cbias")
            nc.vector.memset(cbias[:], C_MAGIC)

            # stack [mx, mb] -> cross-partition -> collective for in_scale
            stk = pers.tile([128, 2], f32, tag="stk")
            nc.vector.tensor_copy(stk[:, 0:1], mx[:])
            nc.vector.tensor_copy(stk[:, 1:2], mb[:])
            par = pers.tile([128, 2], f32, tag="par")
            nc.gpsimd.partition_all_reduce(
                par[:], stk[:], channels=128, reduce_op=bass_isa.ReduceOp.max)
            cin = dram.tile([1, 8], f32, tag="cin")
            cout = dram.tile([1, 8], f32, tag="cout")
            nc.sync.dma_start(cin[0:1, 0:1], par[0:1, 0:1])
            all_reduce_max(cin[0:1, 0:4], cout[0:1, 0:4])
            gm = pers.tile([1, 8], f32, tag="gm")
            nc.sync.dma_start(gm[:], cout[:])

            # scalars: sx = 255/(2*gx); sb = 255/(2*gb); asc = sx*sw;
            # fb = asc/sb.  scb = broadcast [sx, sb, fb]
            g3 = pers.tile([1, 8], f32, tag="g3")
            nc.vector.tensor_copy(g3[0:1, 0:1], gm[0:1, 0:1])
            nc.vector.tensor_copy(g3[0:1, 1:2], par[0:1, 1:2])
            t23 = pers.tile([1, 8], f32, tag="t23")
            nc.vector.tensor_scalar(out=t23[0:1, 0:2], in0=g3[0:1, 0:2],
                                    scalar1=2.0, scalar2=None, op0=Alu.mult)
            rc3 = pers.tile([1, 8], f32, tag="rc3")
            nc.vector.reciprocal(rc3[0:1, 0:2], t23[0:1, 0:2])
            scal = pers.tile([1, 4], f32, tag="scal")
            nc.vector.tensor_scalar(out=scal[0:1, 0:2], in0=rc3[0:1, 0:2],
                                    scalar1=255.0, scalar2=None, op0=Alu.mult)
            sx, sb = scal[0:1, 0:1], scal[0:1, 1:2]
            asc = pers.tile([1, 1], f32, tag="asc")        # accum_scale
            nc.vector.tensor_mul(asc[:], sx, swt[0:1, 2:3])
            rbs = pers.tile([1, 1], f32, tag="rbs")
            nc.vector.reciprocal(rbs[:], sb)
            nc.vector.tensor_mul(scal[0:1, 2:3], asc[:], rbs[:])  # fb
            scb = pers.tile([128, 4], f32, tag="scb")
            nc.gpsimd.partition_broadcast(scb[:], scal[:], channels=128)

            # ---------------- b quantize ----------------
            bq1 = pers.tile([128, 32], f32, tag="bq1")
            nc.vector.tensor_scalar(out=bq1[:], in0=bp[:],
                                    scalar1=scb[:, 1:2], scalar2=C_MAGIC,
                                    op0=Alu.mult, op1=Alu.add)
            bq2 = pers.tile([128, 32], f32, tag="bq2")
            nc.vector.tensor_scalar(out=bq2[:], in0=bq1[:],
                                    scalar1=C_MAGIC, scalar2=127.0,
                                    op0=Alu.subtract, op1=Alu.min)
            bq3 = pers.tile([128, 16], f32, tag="bq3")
            nc.vector.tensor_scalar(out=bq3[:], in0=bq2[:, 0:16],
                                    scalar1=scb[:, 2:3], scalar2=C_MAGIC,
                                    op0=Alu.mult, op1=Alu.add)
            bqf = pers.tile([128, 16], f32, tag="bqf")
            nc.vector.tensor_scalar(out=bqf[:], in0=bq3[:],
                                    scalar1=C_MAGIC, scalar2=None,
                                    op0=Alu.subtract)

            # ---------------- quantize + matmul ----------------
            accs = []
            macc = pers.tile([128, 1], f32, tag="macc")
            xq_ctx = tc.tile_pool(name="xqp", bufs=1, side="right")
            xqp = xq_ctx.__enter__()
            with (
                tc.tile_pool(name="wsp", bufs=2, side="right") as wsp,
                tc.tile_pool(name="wt1p", bufs=2, side="right") as wt1p,
                tc.tile_pool(name="wqp", bufs=5, side="right") as wqp,
            ):
                xs_ctx = tc.tile_pool(name="xsp", bufs=3, side="right")
                xsp = xs_ctx.__enter__()
                xt1_ctx = tc.tile_pool(name="xt1p", bufs=2, side="right")
                xt1p = xt1_ctx.__enter__()

                KH = KT // 2

                wdma = []

                def quant_w(m, pool_ts2=False):
                    eng2 = nc.gpsimd if pool_ts2 else nc.vector
                    wqm = wqp.tile([128, KT, 128], bf16, tag="wq")
                    src = wt_d.ap()[m].rearrange("(k p) j -> p k j", p=128)
                    for h in range(2):
                        ws = wsp.tile([128, KH, 128], f32, tag="ws")
                        wdma.append(nc.sync.dma_start(
                            ws[:], src[:, KH * h:KH * (h + 1), :]))
                        wt1 = wt1p.tile([128, KH, 128], f32, tag="wt1")
                        nc.scalar.activation(wt1[:], ws[:], Act.Identity,
                                             bias=cbias[:, 0:1],
                                             scale=scbw[:, 0:1])
                        eng2.tensor_scalar(
                            out=wqm[:, KH * h:KH * (h + 1), :], in0=wt1[:],
                            scalar1=C_MAGIC, scalar2=127.0,
                            op0=Alu.subtract, op1=Alu.min)
                    return wqm

                xq = [None] * KT

                def quant_x(k):
                    if k < NSTAGE:
                        xs = xf[k]
                    else:
                        xs = xsp.tile([128, BS], f32, tag="xs")
                        xd = nc.sync.dma_start(
                            xs[:], xt_d.ap()[128 * k:128 * (k + 1), :])
                        if len(wdma) >= 4:
                            from concourse.bass import _add_dep_helper
                            _add_dep_helper(
                                xd.ins, wdma[3].ins,
                                reason="W0/W1 staging beats x streams")
                    xqk = xqp.tile([128, BS], bf16, tag=f"xq{k}")
                    if k % 3 == 2 or k < 2:
                        # full-DVE path (exact, offloads ACT)
                        xt1 = xt1p.tile([128, BS], f32, tag="xt1")
                        nc.vector.tensor_scalar(out=xt1[:], in0=xs[:],
                                                scalar1=scb[:, 0:1],
                                                scalar2=C_MAGIC,
                                                op0=Alu.mult, op1=Alu.add)
                        nc.vector.tensor_scalar(out=xqk[:], in0=xt1[:],
                                                scalar1=C_MAGIC, scalar2=127.0,
                                                op0=Alu.subtract, op1=Alu.min)
                    else:
                        # ACT TS1; TS2 on DVE (k%3==0) or Pool (k%3==1)
                        eng2 = nc.gpsimd if k % 3 == 1 else nc.vector
                        xt1 = xt1p.tile([128, BS], f32, tag="xt1")
                        nc.scalar.activation(xt1[:], xs[:], Act.Identity,
                                             bias=cbias[:, 0:1],
                                             scale=scb[:, 0:1])
                        eng2.tensor_scalar(out=xqk[:], in0=xt1[:],
                                           scalar1=C_MAGIC, scalar2=127.0,
                                           op0=Alu.subtract, op1=Alu.min)
                    xq[k] = xqk

                # first x tiles on DVE before the W quants occupy ACT;
                # first W group quantizes during the AR_x bounce window
                quant_x(0)
                quant_x(1)
                wq_pipe = [quant_w(0, True), quant_w(1, True)]
                for k in range(2, KT):
                    quant_x(k)
                xt1_ctx.__exit__(None, None, None)
                xs_ctx.__exit__(None, None, None)
                xf_ctx.__exit__(None, None, None)
                acc_ctx = tc.tile_pool(name="accp", bufs=1)
                accp = acc_ctx.__enter__()

                # m-blocks in groups, k outermost within a group: each xq[k]
                # feeds 2*group matmuls so PE keeps pace with x-quant
                # production (first group bigger to cover the quant crunch)
                GROUPS = [2, 3, 2, 2, 2, 2, 3]
                assert sum(GROUPS) == MT
                m0 = 0
                for gi, gsz in enumerate(GROUPS):
                    if gi + 1 < len(GROUPS):
                        for j in range(GROUPS[gi + 1]):
                            wq_pipe.append(quant_w(m0 + gsz + j,
                                                   pool_ts2=(gi == 0)))
                    gacc = [accp.tile([128, BS], f32, tag=f"acc{m0 + i}",
                                      name=f"acc{m0 + i}")
                            for i in range(gsz)]
                    ps = [psum.tile([128, 512], f32, tag="ps",
                                    name=f"ps{gi}_{i}")
                          for i in range(2 * gsz)]
                    for k in range(KT):
                        for mi in range(gsz):
                            wq_cur = wq_pipe[m0 + mi]
                            for n in range(2):
                                nc.tensor.matmul(
                                    ps[2 * mi + n][:], wq_cur[:, k, :],
                                    xq[k][:, 512 * n:512 * (n + 1)],
                                    start=(k == 0), stop=(k == KT - 1))
                    for mi in range(gsz):
                        acc_m = gacc[mi]
                        for n in range(2):
                            nc.scalar.activation(
                                acc_m[:, 512 * n:512 * (n + 1)],
                                ps[2 * mi + n][:], Act.Identity,
                                bias=bqf[:, m0 + mi:m0 + mi + 1], scale=1.0)
                        rt = stat.tile([128, 1], f32, tag="accr")
                        nc.vector.tensor_reduce(
                            rt[:], acc_m[:], axis=mybir.AxisListType.X,
                            op=Alu.max, apply_absolute_value=True)
                        if m0 + mi == 0:
                            nc.vector.tensor_copy(macc[:], rt[:])
                        else:
                            nc.vector.tensor_max(macc[:], macc[:], rt[:])
                        accs.append(acc_m)
                    m0 += gsz

                # precompute 1/accum_scale off the AR2 critical path
                rac = pers.tile([1, 1], f32, tag="rac")
                nc.vector.reciprocal(rac[:], asc[:])
            xq_ctx.__exit__(None, None, None)

            # ---------------- AR2 + output scalars ----------------
            par2 = pers.tile([128, 1], f32, tag="par2")
            nc.gpsimd.partition_all_reduce(
                par2[:], macc[:], channels=128,
                reduce_op=bass_isa.ReduceOp.max)
            cin2 = dram.tile([1, 8], f32, tag="cin2")
            cout2 = dram.tile([1, 8], f32, tag="cout2")
            nc.sync.dma_start(cin2[0:1, 0:1], par2[0:1, 0:1])
            all_reduce_max(cin2[0:1, 0:4], cout2[0:1, 0:4])
            gm2 = pers.tile([1, 8], f32, tag="gm2")
            nc.sync.dma_start(gm2[:], cout2[:])

            osat = pers.tile([1, 1], f32, tag="osat")  # out_sat
            nc.vector.tensor_mul(osat[:], gm2[0:1, 0:1], rac[:])
            ot2 = pers.tile([1, 1], f32, tag="ot2")
            nc.vector.tensor_scalar(out=ot2[:], in0=osat[:], scalar1=2.0,
                                    scalar2=None, op0=Alu.mult)
            ros = pers.tile([1, 1], f32, tag="ros")
            nc.vector.reciprocal(ros[:], ot2[:])
            oscale = pers.tile([1, 1], f32, tag="oscale")
            nc.vector.tensor_scalar(out=oscale[:], in0=ros[:], scalar1=255.0,
                                    scalar2=None, op0=Alu.mult)
            scal2 = pers.tile([1, 4], f32, tag="scal2")
            nc.vector.tensor_mul(scal2[0:1, 0:1], oscale[:], rac[:])  # rq
            nc.vector.reciprocal(scal2[0:1, 1:2], oscale[:])          # ios
            scb2 = pers.tile([128, 4], f32, tag="scb2")
            nc.gpsimd.partition_broadcast(scb2[:], scal2[:], channels=128)

            # -------- epilogue: requant + dequant (ACT -> DVE -> DVE) -----
            with tc.tile_pool(name="epip", bufs=5) as epip:
                for m in range(MT):
                    e1 = epip.tile([128, BS], f32, tag="e1")
                    nc.scalar.activation(e1[:], accs[m][:], Act.Identity,
                                         bias=cbias[:, 0:1],
                                         scale=scb2[:, 0:1])
                    e2 = epip.tile([128, BS], f32, tag="e2")
                    nc.vector.tensor_scalar(out=e2[:], in0=e1[:],
                                            scalar1=C_P127, scalar2=C_MAGIC,
                                            op0=Alu.min, op1=Alu.subtract)
                    of = epip.tile([128, BS], f32, tag="of")
                    nc.vector.tensor_scalar(out=of[:], in0=e2[:],
                                            scalar1=scb2[:, 1:2], scalar2=None,
                                            op0=Alu.mult)
                    nc.sync.dma_start(
                        out_d.ap()[128 * m:128 * (m + 1), :], of[:])
            acc_ctx.__exit__(None, None, None)

    nc.compile()
    return nc


def _prep_inputs(x, W, b):
    xT = np.ascontiguousarray(x.T)      # [i, b]
    WT = np.ascontiguousarray(W.T)      # [i, o]
    bfull = np.ascontiguousarray(b.reshape(32, 128).T)  # [128, 32]
    in_maps = []
    for core in range(NCORES):
        r, c = divmod(core, GC)
        rho = _roll_tiles(core) * 128
        cols = list(range(16 * c, 16 * c + 16))
        cols += [j for j in range(32) if j not in cols]
        xt = np.roll(xT[:, r * BS:(r + 1) * BS], -rho, axis=0)
        wt = np.roll(WT[:, c * OS:(c + 1) * OS], -rho, axis=0)
        in_maps.append({
            "xt": np.ascontiguousarray(xt),
            "wt": np.ascontiguousarray(
                wt.reshape(K, MT, 128).transpose(1, 0, 2)),
            "bp": np.ascontiguousarray(bfull[:, cols]),
        })
    return in_maps


def kernel(x, W, b):
    from concourse import bass_utils

    x = np.asarray(x, dtype=np.float32)
    W = np.asarray(W, dtype=np.float32)
    b = np.asarray(b, dtype=np.float32)
    assert x.shape == (B, K) and W.shape == (O, K) and b.shape == (O,)

    if "nc" not in _CACHE:
        _CACHE["nc"] = _build_nc()
    nc = _CACHE["nc"]

    in_maps = _prep_inputs(x, W, b)
    res = bass_utils.run_bass_kernel_spmd(
        nc, in_maps, core_ids=list(range(NCORES)))
    _CACHE["last_results"] = res

    full = np.empty((B, O), dtype=np.float32)
    for core in range(NCORES):
        r, c = divmod(core, GC)
        blk = res.results[core]["out"]          # [OS, BS] = [o, b]
        full[r * BS:(r + 1) * BS, c * OS:(c + 1) * OS] = blk.T
    return full
